# revision 1
# baseline (speedup 1.0000x reference)
"""Trainium2 Bass kernel for CalibrationFreeFP8Linear.

Computes: quantize x and w to fp8-e4m3 with EMA-updated dynamic absmax
scales, fp8 matmul (fp32 accumulate), dequantize, cast to bf16.

Sharding: data-parallel over the 16384 (B*S) rows of x across 8 cores;
weight replicated. The x absmax needs a global max -> AllGather(max).

Host side pre-transposes both operands to K-major layout ([K, M] / [K, N])
so the tensor engine (which contracts over the partition axis for both
operands) gets contiguous DMA loads with no on-device transpose.

Pipeline (per core), engineered so the tensor engine starts as early as
the data dependencies allow (w fully loaded -> absmax -> scale -> first
fp8 chunks):
  1. x loaded first with full DMA priority (fine-grained transfers so
     absmax can pipeline under the load), w second.
  2. absmax with one-pass abs_max+accum tensor_scalar on DVE (4x rate),
     a few chunks on the Pool engine.
  3. x global absmax: tiny AllGather triggered ~immediately after the x
     load drains (stat DMAs ride SWDGE/ACT rings so they never queue
     behind the bulk loads).
  4. quantize split across DVE/ACT/Pool; fp8 DoubleRow matmuls m->kk->n,
     dequant epilogue split ACT/DVE, stores on the sync ring (last m-tile
     split across queues to shrink the output tail).
"""

import numpy as np
import ml_dtypes

import concourse.bass as bass
import concourse.mybir as mybir
import concourse.tile as tile
from concourse import bacc, bass_isa
from concourse.bass import ts
from concourse.bass_utils import run_bass_kernel_spmd

FP8_MAX = 448.0
EMA = 0.9
N_CORES = 8
P = 128

# Full problem shapes (hardcoded; kernel.py must be self-contained).
B, S, K, N = 4, 4096, 2048, 2048
M_PER_CORE = (B * S) // N_CORES  # 2048

# ---- tuning flags
# absmax mode, in fallback order (first one that compiles is used):
#   tt_absmax:  DVE running-max accumulator via tensor_tensor(abs_max) @2x
#   act_abs_tt: ACT abs pass + DVE tensor_tensor(max) accumulator @2x
#   reduce:     plain 1x tensor_reduce per chunk (known-good baseline)
ABSMAX_MODES = ("act_abs_tt", "reduce")
POOL_QUANT = 0           # Pool elementwise is ~40x slow AND stalls DVE: keep 0
DMA_SUBP = 128           # partitions per bulk-load transfer (128 = whole chunk)
WARMUP_MM = 12           # junk fp8 matmuls (on wf chunk 0) to ramp the PE clock
SPLIT_LAST_STORE = True  # split last m-tile stores across queues


def build_nc(M, K, N, n_cores=N_CORES, absmax_mode="act_abs_tt"):
    """Build the SPMD Bass program for one core's [M, K] @ [K, N]^T shard.

    DRAM inputs (per core): xt [K, M] bf16, wt [K, N] bf16 (both K-major),
    in_s [1] f32, w_s [1] f32. Output: out [M, N] bf16.
    """
    dt = mybir.dt
    KT = K // P            # k-subtiles
    MT = M // P            # m-tiles
    N_TILE = min(512, N)
    NT = N // N_TILE
    assert K % P == 0 and M % P == 0 and N % N_TILE == 0
    assert KT % 2 == 0, "DoubleRow needs an even number of k-subtiles"
    SUBP = DMA_SUBP
    NSP = P // SUBP
    assert P % SUBP == 0

    nc = bacc.Bacc(
        "TRN2",
        target_bir_lowering=False,
        debug=False,
        num_devices=n_cores,
    )

    xt = nc.dram_tensor("xt", [K, M], dt.bfloat16, kind="ExternalInput").ap()
    wt = nc.dram_tensor("wt", [K, N], dt.bfloat16, kind="ExternalInput").ap()
    in_s = nc.dram_tensor("in_s", [1], dt.float32, kind="ExternalInput").ap()
    w_s = nc.dram_tensor("w_s", [1], dt.float32, kind="ExternalInput").ap()
    out = nc.dram_tensor("out", [M, N], dt.bfloat16, kind="ExternalOutput").ap()

    # K-major views of the DRAM tensors: k = ko*P + p
    xt_v = xt.rearrange("(ko p) m -> p ko m", p=P)
    wt_v = wt.rearrange("(ko p) n -> p ko n", p=P)
    out_v = out.rearrange("(mo p) n -> p mo n", p=P)

    rg = [list(range(n_cores))]
    MX = mybir.AluOpType.max
    MN = mybir.AluOpType.min
    ABSMX = mybir.AluOpType.abs_max
    AXX = mybir.AxisListType.X

    with tile.TileContext(nc) as tc:
        with (
            tc.tile_pool(name="stats", bufs=1) as stats,
            tc.tile_pool(name="dram", bufs=1, space="DRAM") as dram,
            tc.tile_pool(name="xb_pool", bufs=1) as xb_pool,
            tc.tile_pool(name="wb_pool", bufs=1) as wb_pool,
            tc.tile_pool(name="wf_pool", bufs=1) as wf_pool,
            tc.tile_pool(name="xf_pool", bufs=1) as xf_pool,
            tc.tile_pool(name="psum", bufs=max(1, 8 // NT), space="PSUM") as psum,
            tc.tile_pool(name="outp", bufs=3) as outp,
        ):
            # ---- absmax machinery
            # act_abs_tt balances two pipelines: the first ACT_SPLIT chunks
            # go ACT-abs (1.9us) + DVE tensor_tensor max-accumulate (1.24us);
            # the rest go direct DVE 1x tensor_reduce (2.2us). Balancing the
            # two engine chains minimizes the absmax makespan per tensor.
            CH = max(M, N)
            ACT_SPLIT = (3 * KT) // 4 if absmax_mode == "act_abs_tt" else 0
            if absmax_mode == "act_abs_tt":
                acc = stats.tile([P, CH], dt.bfloat16)
                ab0 = stats.tile([P, CH], dt.bfloat16)
                ab1 = stats.tile([P, CH], dt.bfloat16)
            rc_x = stats.tile([P, KT], dt.float32)
            rc_w = stats.tile([P, KT], dt.float32)

            def absmax_op(src2d, rc, j):
                """fold one [P, F] bf16 chunk into the running absmax state."""
                F = src2d.shape[-1]
                if absmax_mode == "act_abs_tt" and j < ACT_SPLIT:
                    ab = ab0 if j % 2 == 0 else ab1
                    nc.scalar.activation(
                        ab[:, :F], src2d, mybir.ActivationFunctionType.Abs
                    )
                    if j == 0:
                        # max(|c0|, |c0|) = |c0| -- avoids a tensor_copy
                        nc.vector.tensor_tensor(acc[:, :F], ab[:, :F], ab[:, :F], MX)
                    else:
                        nc.vector.tensor_tensor(acc[:, :F], acc[:, :F], ab[:, :F], MX)
                else:
                    nc.vector.tensor_reduce(
                        rc[:, j : j + 1], src2d, axis=AXX, op=MX,
                        apply_absolute_value=True,
                    )

            def absmax_final(rc, F, name):
                """combine the running state -> [P,1] absmax (on DVE)."""
                amax = stats.tile([P, 1], dt.float32, name=f"amax_{name}")
                if absmax_mode == "act_abs_tt":
                    nc.vector.tensor_reduce(amax, acc[:, :F], axis=AXX, op=MX)
                    if ACT_SPLIT < KT:
                        t_rc = stats.tile([P, 1], dt.float32, name=f"trc_{name}")
                        nc.vector.tensor_reduce(
                            t_rc, rc[:, ACT_SPLIT:KT], axis=AXX, op=MX
                        )
                        nc.vector.tensor_tensor(amax, amax, t_rc, MX)
                else:
                    nc.vector.tensor_reduce(amax, rc, axis=AXX, op=MX)
                return amax

            # EMA scale inputs: tiny loads on the Pool SWDGE ring so they
            # never contend with the bulk loads on the sync HWDGE ring.
            prev_s = stats.tile([1, 2], dt.float32)
            nc.gpsimd.dma_start(prev_s[:, 0:1], in_s.rearrange("(o p) -> p o", p=1))
            nc.gpsimd.dma_start(prev_s[:, 1:2], w_s.rearrange("(o p) -> p o", p=1))
            prev_b = stats.tile([P, 2], dt.float32)
            nc.gpsimd.partition_broadcast(prev_b, prev_s, channels=P)

            # ---- bulk loads: x with full priority, then w, on the sync ring.
            # Each k-chunk is split into NSP partition-slices so chunk
            # completions stagger and absmax pipelines under the load.
            xb = xb_pool.tile([P, KT, M], dt.bfloat16)
            wb = wb_pool.tile([P, KT, N], dt.bfloat16)
            for j in range(KT):
                for pp in range(NSP):
                    sl = slice(pp * SUBP, (pp + 1) * SUBP)
                    nc.sync.dma_start(xb[sl, ts(j, 1)], xt_v[sl, ts(j, 1)])
            for j in range(KT):
                for pp in range(NSP):
                    sl = slice(pp * SUBP, (pp + 1) * SUBP)
                    nc.sync.dma_start(wb[sl, ts(j, 1)], wt_v[sl, ts(j, 1)])

            # ---- absmax: x chunks then w chunks (arrival order)
            for j in range(KT):
                src = xb[:, ts(j, 1)].rearrange("p a b -> p (a b)")
                absmax_op(src, rc_x, j)
            amax_x = absmax_final(rc_x, M, "x")

            # Pool: cross-partition reduce, stat store, collective trigger
            amax_x_b = stats.tile([P, 1], dt.float32)
            nc.gpsimd.partition_all_reduce(
                amax_x_b, amax_x, channels=P, reduce_op=bass_isa.ReduceOp.max
            )
            cc_in = dram.tile([P], dt.float32)
            cc_in_v = cc_in.rearrange("(o p) -> p o", p=P)
            nc.gpsimd.dma_start(cc_in_v, amax_x_b)
            if n_cores > 1:
                cc_out = dram.tile([n_cores * P], dt.float32, addr_space="Shared")
                nc.gpsimd.collective_compute(
                    "AllGather",
                    mybir.AluOpType.bypass,
                    replica_groups=rg,
                    ins=[cc_in.opt()],
                    outs=[cc_out.opt()],
                )
                gath = stats.tile([1, n_cores], dt.float32)
                cc_lead = cc_out.rearrange("(c p) -> p c", p=P)[0:1, :]
            else:
                gath = None

            # w absmax (chunks land after x; the acc state is reused -- the
            # x side fully drains on DVE before the first w op runs)
            for j in range(KT):
                src = wb[:, ts(j, 1)].rearrange("p a b -> p (a b)")
                absmax_op(src, rc_w, j)
            amax_w = absmax_final(rc_w, N, "w")
            if gath is not None:
                # Only the 8 leader values matter: strided 4B gather on the
                # ACT HWDGE ring -- emitted after the w abs passes so the ACT
                # sequencer never stalls on the collective before them.
                nc.scalar.dma_start(gath, cc_lead)
            amax_w_b = stats.tile([P, 1], dt.float32)
            nc.gpsimd.partition_all_reduce(
                amax_w_b, amax_w, channels=P, reduce_op=bass_isa.ReduceOp.max
            )

            def ema_scale(amax_col, prev_col, name):
                t = stats.tile([P, 1], dt.float32, name=f"t_{name}")
                nc.vector.tensor_scalar_add(t, amax_col, 1e-12)
                nc.vector.reciprocal(t, t)
                nc.vector.tensor_scalar_mul(t, t, FP8_MAX)
                nc.vector.tensor_scalar(
                    t, t, 1e-6, 1e6, mybir.AluOpType.max, mybir.AluOpType.min
                )
                s = stats.tile([P, 1], dt.float32, name=f"s_{name}")
                nc.vector.tensor_scalar_mul(s, t, 1.0 - EMA)
                t2 = stats.tile([P, 1], dt.float32, name=f"t2_{name}")
                nc.vector.tensor_scalar_mul(t2, prev_col, EMA)
                nc.vector.tensor_add(s, s, t2)
                return s

            # w-side scale first on DVE (w gates the matmul start), then the
            # collective-gated x-side chain.
            s_w = ema_scale(amax_w_b, prev_b[:, 1:2], "w")
            if gath is not None:
                red = stats.tile([1, 1], dt.float32)
                nc.vector.tensor_reduce(red, gath, axis=AXX, op=MX)
                amax_x_g = stats.tile([P, 1], dt.float32)
                nc.gpsimd.partition_broadcast(amax_x_g, red, channels=P)
            else:
                amax_x_g = amax_x_b
            s_x = ema_scale(amax_x_g, prev_b[:, 0:1], "x")

            # ---- quantize: greedy 3-engine split, kk-ascending per engine
            xf = xf_pool.tile([P, KT, M], dt.float8e4)
            wf = wf_pool.tile([P, KT, N], dt.float8e4)

            seq = [("w", j) for j in range(KT)] + [("x", j) for j in range(KT)]
            rates = {"dve": 0.82, "act": 1.10, "pool": 40.0}
            clocks = {"dve": 0.35, "act": 0.0, "pool": 0.5}
            budget = {"dve": 10**9, "act": 10**9, "pool": POOL_QUANT}
            assign = {"dve": [], "act": [], "pool": []}
            for item in seq:
                eng = min(
                    (e for e in rates if budget[e] > len(assign[e])),
                    key=lambda e: clocks[e] + rates[e],
                )
                assign[eng].append(item)
                clocks[eng] += rates[eng]

            def q_src_dst(item):
                t, j = item
                if t == "x":
                    return xb[:, ts(j, 1)], xf[:, ts(j, 1)], s_x
                return wb[:, ts(j, 1)], wf[:, ts(j, 1)], s_w

            for item in assign["dve"]:
                src, dst, s = q_src_dst(item)
                nc.vector.tensor_scalar_mul(dst, src, s)
            for item in assign["act"]:
                src, dst, s = q_src_dst(item)
                nc.scalar.mul(dst.rearrange("p a b -> p (a b)"),
                              src.rearrange("p a b -> p (a b)"), mul=s)
            for item in assign["pool"]:
                src, dst, s = q_src_dst(item)
                nc.gpsimd.tensor_scalar_mul(dst, src, s)

            # inv = 1 / (s_x * s_w) for the output dequant
            inv = stats.tile([P, 1], dt.float32)
            nc.vector.tensor_mul(inv, s_x, s_w)
            nc.vector.reciprocal(inv, inv)

            # ---- fp8 DoubleRow matmul + dequant epilogue
            for m in range(MT):
                pts = [
                    psum.tile([P, N_TILE], dt.float32, name=f"pt{n}") for n in range(NT)
                ]
                if m == 0 and WARMUP_MM > 0:
                    # junk fp8 matmuls on the first quantized w chunk to ramp
                    # the PE clock just before the real phase starts;
                    # overwritten by the kk=0 start=True matmul.
                    for i in range(WARMUP_MM):
                        nc.tensor.matmul(
                            pts[0], wf[:, 0, 0:P], wf[:, 0, ts(0, N_TILE)],
                            start=True, stop=True, skip_group_check=True,
                        )
                for kk in range(KT // 2):
                    for n in range(NT):
                        nc.tensor.matmul(
                            pts[n],
                            xf[:, 2 * kk : 2 * kk + 2, ts(m, P)],
                            wf[:, 2 * kk : 2 * kk + 2, ts(n, N_TILE)],
                            start=(kk == 0),
                            stop=(kk == KT // 2 - 1),
                            perf_mode=mybir.MatmulPerfMode.DoubleRow,
                        )
                for n in range(NT):
                    out_mn = outp.tile([P, N_TILE], dt.bfloat16, name="out_mn")
                    # split the dequant epilogue across ACT and DVE
                    if n % 2 == 0:
                        nc.scalar.mul(out_mn, pts[n], mul=inv)
                    else:
                        nc.vector.tensor_scalar_mul(out_mn, pts[n], inv)
                    if SPLIT_LAST_STORE and m == MT - 1:
                        for pp in range(4):
                            sl = slice(pp * 32, (pp + 1) * 32)
                            nc.sync.dma_start(
                                out_v[sl, m, ts(n, N_TILE)], out_mn[sl, :]
                            )
                    else:
                        nc.sync.dma_start(out_v[:, m, ts(n, N_TILE)], out_mn)

    nc.compile()
    return nc


_NC_CACHE = {}
_WORKING_MODE = [0]


def _get_nc(M, K, N, n_cores=N_CORES, mode_idx=0):
    key = (M, K, N, n_cores, mode_idx)
    if key not in _NC_CACHE:
        _NC_CACHE[key] = build_nc(
            M, K, N, n_cores, absmax_mode=ABSMAX_MODES[mode_idx]
        )
    return _NC_CACHE[key]


def run_sharded(x2d, weight, input_scale, weight_scale, n_cores=N_CORES, trace=False):
    """x2d: [rows, K] bf16, weight: [N, K] bf16. Returns ([rows, N] bf16, result)."""
    rows, k = x2d.shape
    n = weight.shape[0]
    m_per = rows // n_cores
    wt = np.ascontiguousarray(weight.T)  # [K, N]
    in_s = np.asarray(input_scale, dtype=np.float32).reshape(1)
    w_s = np.asarray(weight_scale, dtype=np.float32).reshape(1)
    in_maps = []
    for i in range(n_cores):
        xt_i = np.ascontiguousarray(x2d[i * m_per : (i + 1) * m_per].T)  # [K, M]
        in_maps.append({"xt": xt_i, "wt": wt, "in_s": in_s, "w_s": w_s})

    last_err = None
    start_idx = _WORKING_MODE[0]
    for mode_idx in range(start_idx, len(ABSMAX_MODES)):
        try:
            nc = _get_nc(m_per, k, n, n_cores, mode_idx)
            res = run_bass_kernel_spmd(
                nc, in_maps, core_ids=list(range(n_cores)), trace=trace
            )
            _WORKING_MODE[0] = mode_idx
            break
        except Exception as e:  # fall back to the next absmax mode
            last_err = e
            _NC_CACHE.pop((m_per, k, n, n_cores, mode_idx), None)
            if mode_idx == len(ABSMAX_MODES) - 1:
                raise
    out = np.concatenate([res.results[i]["out"] for i in range(n_cores)], axis=0)
    return out, res


def kernel(x, weight, input_scale, weight_scale):
    x = np.asarray(x)
    weight = np.asarray(weight)
    b, s, k = x.shape
    x2d = np.ascontiguousarray(x.reshape(b * s, k))
    out, _ = run_sharded(x2d, weight, input_scale, weight_scale)
    return out.reshape(b, s, weight.shape[0]).astype(ml_dtypes.bfloat16)



# revision 2
# speedup vs baseline: 1.0786x; 1.0786x over previous
"""Trainium2 Bass kernel for CalibrationFreeFP8Linear.

Computes: quantize x and w to fp8-e4m3 with EMA-updated dynamic absmax
scales, fp8 matmul (fp32 accumulate), dequantize, cast to bf16.

Sharding: data-parallel over the 16384 (B*S) rows of x across 8 cores;
weight replicated.  The absmax for BOTH tensors is combined in a single
AllGather: each core contributes its per-partition x absmax plus the
absmax of a 2-chunk shard of w (w is replicated, so the 16 w chunks are
absmax-scanned cooperatively, 2 per core).  The host rotates the K-chunk
order of both x and w by 2*core so the SPMD program always scans chunks
0..1 of w -- identical program on every core.

Host side packs operands per-partition-contiguous ([128, KT, M] with
partition p holding k-rows {j*128+p}) so bulk loads issue as 1 MiB
transfers (~340 GB/s vs ~250 GB/s for 512 KiB) in FIFO order, giving
staggered chunk arrivals that the absmax chain pipelines under.

Pipeline (per core):
  1. loads on the sync HWDGE ring: w chunks 0-1 (the absmax shard),
     x chunks 0-15, w chunks 2-15.  1 MiB (2-chunk) transfers.
  2. absmax via DVE tensor_tensor_reduce chains (one op per chunk,
     max-accumulator threaded through `scalar`), pipelined under loads.
  3. one AllGather of [x_absmax[128] | w_shard_absmax[128]] per core;
     readback on the ACT ring, [1,2,1024] DVE reduce, EMA scale math
     vectorized on [1,2], one partition_broadcast of (s_x, s_w, inv).
  4. quantize chunks in matmul consumption order, split DVE/ACT.
  5. fp8 DoubleRow matmuls m->kk->n with 2 m-tiles of PSUM in flight:
     the PE starts as soon as the first quantized pair lands (~40us)
     and is arrival-paced until w finishes, then runs flat out.
  6. dequant epilogue alternating ACT/DVE, stores on the sync ring
     (last m-tile split across the sync+ACT rings to shrink the tail).
"""

import numpy as np
import ml_dtypes

import concourse.bass as bass
import concourse.mybir as mybir
import concourse.tile as tile
from concourse import bacc, bass_isa
from concourse.bass import ts
from concourse.bass_utils import run_bass_kernel_spmd

FP8_MAX = 448.0
EMA = 0.9
N_CORES = 8
P = 128

# Full problem shapes (hardcoded; kernel.py must be self-contained).
B, S, K, N = 4, 4096, 2048, 2048
M_PER_CORE = (B * S) // N_CORES  # 2048

# ---- tuning flags
# absmax mode, in fallback order (first one that compiles is used):
#   ttr:     DVE tensor_tensor_reduce chain (1 op/chunk, accum threaded)
#   reduce:  plain 1x tensor_reduce per chunk + final reduce (known-good)
ABSMAX_MODES = ("ttr", "reduce")
LOAD_GROUP = 2           # K-chunks per bulk-load transfer (2 -> 1 MiB)
W_SHARD = 2              # w chunks absmax-scanned per core (16/8)
QUANT_RATES = {"dve": 1.2, "act": 2.0}  # us/chunk, for the greedy split


def build_nc(M, Kd, Nd, n_cores=N_CORES, absmax_mode="ttr"):
    """Build the SPMD Bass program for one core's [M, Kd] @ [Kd, Nd]^T shard.

    DRAM inputs (per core, chunk-rotated by 2*core host-side):
      xt [P, KT*M] bf16   xt[p, j*M+m] = x[m, k=(j+2c)%KT*P+p]
      wt [P, KT*Nd] bf16  wt[p, j*Nd+n] = w[n, same k]
      in_s [1] f32, w_s [1] f32.
    Output: out [P, MT*Nd] bf16, out[p, mo*Nd+n] = out_row(mo*P+p, n).
    """
    dt = mybir.dt
    KT = Kd // P           # 16 k-chunks
    MT = M // P            # 16 m-tiles
    N_TILE = min(512, Nd)
    NT = Nd // N_TILE      # 4 n-tiles
    assert Kd % P == 0 and M % P == 0 and Nd % N_TILE == 0
    assert KT % 2 == 0, "DoubleRow needs an even number of k-subtiles"
    G = LOAD_GROUP
    assert KT % G == 0

    nc = bacc.Bacc(
        "TRN2",
        target_bir_lowering=False,
        debug=False,
        num_devices=n_cores,
    )

    xt = nc.dram_tensor("xt", [P, KT * M], dt.bfloat16, kind="ExternalInput").ap()
    wt = nc.dram_tensor("wt", [P, KT * Nd], dt.bfloat16, kind="ExternalInput").ap()
    in_s = nc.dram_tensor("in_s", [1], dt.float32, kind="ExternalInput").ap()
    w_s = nc.dram_tensor("w_s", [1], dt.float32, kind="ExternalInput").ap()
    out = nc.dram_tensor("out", [P, MT * Nd], dt.bfloat16, kind="ExternalOutput").ap()

    xt_v = xt.rearrange("p (j m) -> p j m", j=KT)
    wt_v = wt.rearrange("p (j n) -> p j n", j=KT)
    out_v = out.rearrange("p (mo n) -> p mo n", mo=MT)

    rg = [list(range(n_cores))]
    MX = mybir.AluOpType.max
    MN = mybir.AluOpType.min
    ABSMX = mybir.AluOpType.abs_max
    AXX = mybir.AxisListType.X

    with tile.TileContext(nc) as tc:
        with (
            tc.tile_pool(name="stats", bufs=1) as stats,
            tc.tile_pool(name="dram", bufs=1, space="DRAM") as dram,
            tc.tile_pool(name="xb_pool", bufs=1) as xb_pool,
            tc.tile_pool(name="wb_pool", bufs=1) as wb_pool,
            tc.tile_pool(name="wf_pool", bufs=1) as wf_pool,
            tc.tile_pool(name="xf_pool", bufs=1) as xf_pool,
            tc.tile_pool(name="psum", bufs=max(1, 8 // NT), space="PSUM") as psum,
            tc.tile_pool(name="outp", bufs=3) as outp,
        ):
            # ---- EMA prev scales: tiny loads on the gpsimd SWDGE ring so
            # they never contend with bulk loads; p9 = 0.9*prev precomputed.
            pv = stats.tile([1, 2], dt.float32)
            nc.gpsimd.dma_start(pv[:, 0:1], in_s.rearrange("(o p) -> p o", p=1))
            nc.gpsimd.dma_start(pv[:, 1:2], w_s.rearrange("(o p) -> p o", p=1))
            p9 = stats.tile([1, 2], dt.float32)
            nc.vector.tensor_scalar_mul(p9, pv, EMA)

            # ---- bulk loads, FIFO on the sync HWDGE ring:
            # w shard (chunks 0..1) -> x chunks -> w rest.  1 MiB each.
            xb = xb_pool.tile([P, KT, M], dt.bfloat16)
            wb = wb_pool.tile([P, KT, Nd], dt.bfloat16)
            nc.sync.dma_start(wb[:, 0:W_SHARD], wt_v[:, 0:W_SHARD])
            for g in range(KT // G):
                nc.sync.dma_start(xb[:, ts(g, G)], xt_v[:, ts(g, G)])
            for g in range(W_SHARD // G, KT // G):
                nc.sync.dma_start(wb[:, ts(g, G)], wt_v[:, ts(g, G)])

            # ---- absmax: s_amax[:,0] = x per-partition absmax,
            #              s_amax[:,1] = w-shard per-partition absmax.
            s_amax = stats.tile([P, 2], dt.float32)
            if absmax_mode == "ttr":
                scratch = stats.tile([P, max(M, Nd)], dt.bfloat16)
                aping = stats.tile([P, 2], dt.float32)

                def absmax_chain(src_tile, js, accum_final, name):
                    prev = 0.0
                    for idx, j in enumerate(js):
                        src = src_tile[:, ts(j, 1)].rearrange("p a b -> p (a b)")
                        F = src.shape[-1]
                        last = idx == len(js) - 1
                        nxt = accum_final if last else aping[:, idx % 2 : idx % 2 + 1]
                        nc.vector.tensor_tensor_reduce(
                            out=scratch[:, :F],
                            in0=src,
                            in1=src,
                            scale=1.0,
                            scalar=prev,
                            op0=ABSMX,
                            op1=MX,
                            accum_out=nxt,
                        )
                        prev = nxt

                absmax_chain(wb, range(W_SHARD), s_amax[:, 1:2], "w")
                absmax_chain(xb, range(KT), s_amax[:, 0:1], "x")
            else:  # "reduce" fallback
                rc_x = stats.tile([P, KT], dt.float32)
                rc_w = stats.tile([P, W_SHARD], dt.float32)
                for j in range(W_SHARD):
                    src = wb[:, ts(j, 1)].rearrange("p a b -> p (a b)")
                    nc.vector.tensor_reduce(
                        rc_w[:, j : j + 1], src, axis=AXX, op=MX,
                        apply_absolute_value=True,
                    )
                for j in range(KT):
                    src = xb[:, ts(j, 1)].rearrange("p a b -> p (a b)")
                    nc.vector.tensor_reduce(
                        rc_x[:, j : j + 1], src, axis=AXX, op=MX,
                        apply_absolute_value=True,
                    )
                nc.vector.tensor_reduce(s_amax[:, 1:2], rc_w, axis=AXX, op=MX)
                nc.vector.tensor_reduce(s_amax[:, 0:1], rc_x, axis=AXX, op=MX)

            # ---- stat out + AllGather.  cc_in layout [t, p]: t=0 x, t=1 w.
            cc_in = dram.tile([2 * P], dt.float32)
            cc_in_v = cc_in.rearrange("(t p) -> p t", p=P)
            nc.scalar.dma_start(cc_in_v[:, 0:1], s_amax[:, 0:1])
            nc.scalar.dma_start(cc_in_v[:, 1:2], s_amax[:, 1:2])
            if n_cores > 1:
                cc_out = dram.tile([n_cores * 2 * P], dt.float32, addr_space="Shared")
                nc.gpsimd.collective_compute(
                    "AllGather",
                    mybir.AluOpType.bypass,
                    replica_groups=rg,
                    ins=[cc_in.opt()],
                    outs=[cc_out.opt()],
                )
                # readback on the ACT ring: [1, t, (c p)] so one DVE reduce
                # yields [1, 2] = (amax_x, amax_w).
                gath = stats.tile([1, 2, n_cores * P], dt.float32)
                cc_rv = cc_out.rearrange("(o c t p) -> o t c p", o=1, t=2, p=P)
                nc.scalar.dma_start(gath, cc_rv)
                g2 = stats.tile([1, 2], dt.float32)
                nc.vector.tensor_reduce(g2, gath, axis=AXX, op=MX)
            else:
                g2 = stats.tile([1, 2], dt.float32)
                gath1 = stats.tile([1, 2, P], dt.float32)
                nc.scalar.dma_start(
                    gath1, cc_in.rearrange("(o t p) -> o t p", o=1, t=2)
                )
                nc.vector.tensor_reduce(g2, gath1, axis=AXX, op=MX)

            # ---- EMA scales, vectorized over (x, w) on [1, 2]:
            # s = 0.9*prev + 0.1*clip(448/(amax+1e-12), 1e-6, 1e6)
            nc.vector.tensor_scalar_add(g2, g2, 1e-12)
            nc.vector.reciprocal(g2, g2)
            nc.vector.tensor_scalar_mul(g2, g2, FP8_MAX)
            nc.vector.tensor_scalar(g2, g2, 1e-6, 1e6, MX, MN)
            nc.vector.tensor_scalar_mul(g2, g2, float(1.0 - EMA))
            sf = stats.tile([1, 3], dt.float32)
            nc.vector.tensor_add(sf[:, 0:2], g2, p9)
            # inv = 1 / (s_x * s_w) for the output dequant
            nc.vector.tensor_mul(sf[:, 2:3], sf[:, 0:1], sf[:, 1:2])
            nc.vector.reciprocal(sf[:, 2:3], sf[:, 2:3])
            sb = stats.tile([P, 3], dt.float32)
            nc.gpsimd.partition_broadcast(sb, sf, channels=P)
            s_x, s_w, inv = sb[:, 0:1], sb[:, 1:2], sb[:, 2:3]

            # ---- quantize in matmul consumption order (pair-k blocks),
            # greedy split across DVE / ACT.
            xf = xf_pool.tile([P, KT, M], dt.float8e4)
            wf = wf_pool.tile([P, KT, Nd], dt.float8e4)

            seq = []
            for k in range(KT // 2):
                seq += [("w", 2 * k), ("w", 2 * k + 1), ("x", 2 * k), ("x", 2 * k + 1)]
            clocks = {"dve": 0.0, "act": 0.0}
            assign = []
            for item in seq:
                eng = min(clocks, key=lambda e: clocks[e] + QUANT_RATES[e])
                clocks[eng] += QUANT_RATES[eng]
                assign.append((eng, item))
            for eng, (t, j) in assign:
                if t == "x":
                    src, dst, s = xb[:, ts(j, 1)], xf[:, ts(j, 1)], s_x
                else:
                    src, dst, s = wb[:, ts(j, 1)], wf[:, ts(j, 1)], s_w
                if eng == "dve":
                    nc.vector.tensor_scalar_mul(dst, src, s)
                else:
                    nc.scalar.mul(dst.rearrange("p a b -> p (a b)"),
                                  src.rearrange("p a b -> p (a b)"), mul=s)

            # ---- fp8 DoubleRow matmul + dequant epilogue.
            # psum pool bufs=2 -> two m-tiles in flight; the PE starts on
            # quantized pair 0 and is arrival-paced until all pairs land.
            for m in range(MT):
                pts = [
                    psum.tile([P, N_TILE], dt.float32, name=f"pt{n}") for n in range(NT)
                ]
                for kk in range(KT // 2):
                    for n in range(NT):
                        nc.tensor.matmul(
                            pts[n],
                            xf[:, 2 * kk : 2 * kk + 2, ts(m, P)],
                            wf[:, 2 * kk : 2 * kk + 2, ts(n, N_TILE)],
                            start=(kk == 0),
                            stop=(kk == KT // 2 - 1),
                            perf_mode=mybir.MatmulPerfMode.DoubleRow,
                        )
                for n in range(NT):
                    out_mn = outp.tile([P, N_TILE], dt.bfloat16, name="out_mn")
                    # split the dequant epilogue across ACT and DVE
                    if n % 2 == 0:
                        nc.scalar.mul(out_mn, pts[n], mul=inv)
                    else:
                        nc.vector.tensor_scalar_mul(out_mn, pts[n], inv)
                    if m == MT - 1 and n % 2 == 1:
                        # last m-tile: alternate stores onto the ACT ring
                        # to halve the store tail
                        nc.scalar.dma_start(out_v[:, m, ts(n, N_TILE)], out_mn)
                    else:
                        nc.sync.dma_start(out_v[:, m, ts(n, N_TILE)], out_mn)

    nc.compile()
    return nc


_NC_CACHE = {}
_WORKING_MODE = [0]


def _get_nc(M, Kd, Nd, n_cores=N_CORES, mode_idx=0):
    key = (M, Kd, Nd, n_cores, mode_idx)
    if key not in _NC_CACHE:
        _NC_CACHE[key] = build_nc(
            M, Kd, Nd, n_cores, absmax_mode=ABSMAX_MODES[mode_idx]
        )
    return _NC_CACHE[key]


def _pack_rotated(a_km, KT, rot):
    """[K, F] -> [P, KT*F] with chunk j holding original chunk (j+rot)%KT,
    per-partition contiguous."""
    Kd, F = a_km.shape
    a = a_km.reshape(KT, P, F)
    if rot:
        a = np.roll(a, -rot, axis=0)
    return np.ascontiguousarray(a.transpose(1, 0, 2)).reshape(P, KT * F)


def run_sharded(x2d, weight, input_scale, weight_scale, n_cores=N_CORES, trace=False):
    """x2d: [rows, K] bf16, weight: [N, K] bf16. Returns ([rows, N] bf16, result)."""
    rows, k = x2d.shape
    n = weight.shape[0]
    m_per = rows // n_cores
    KT = k // P
    MT = m_per // P
    wT = np.ascontiguousarray(weight.T)  # [K, N]
    in_s = np.asarray(input_scale, dtype=np.float32).reshape(1)
    w_s = np.asarray(weight_scale, dtype=np.float32).reshape(1)
    in_maps = []
    for i in range(n_cores):
        rot = (2 * i) % KT
        xt_i = _pack_rotated(
            np.ascontiguousarray(x2d[i * m_per : (i + 1) * m_per].T), KT, rot
        )
        wt_i = _pack_rotated(wT, KT, rot)
        in_maps.append({"xt": xt_i, "wt": wt_i, "in_s": in_s, "w_s": w_s})

    last_err = None
    start_idx = _WORKING_MODE[0]
    for mode_idx in range(start_idx, len(ABSMAX_MODES)):
        try:
            nc = _get_nc(m_per, k, n, n_cores, mode_idx)
            res = run_bass_kernel_spmd(
                nc, in_maps, core_ids=list(range(n_cores)), trace=trace
            )
            _WORKING_MODE[0] = mode_idx
            break
        except Exception as e:  # fall back to the next absmax mode
            last_err = e
            _NC_CACHE.pop((m_per, k, n, n_cores, mode_idx), None)
            if mode_idx == len(ABSMAX_MODES) - 1:
                raise
    outs = []
    for i in range(n_cores):
        o = res.results[i]["out"].reshape(P, MT, n)
        outs.append(o.transpose(1, 0, 2).reshape(m_per, n))
    out = np.concatenate(outs, axis=0)
    return out, res


def kernel(x, weight, input_scale, weight_scale):
    x = np.asarray(x)
    weight = np.asarray(weight)
    b, s, k = x.shape
    x2d = np.ascontiguousarray(x.reshape(b * s, k))
    out, _ = run_sharded(x2d, weight, input_scale, weight_scale)
    return out.reshape(b, s, weight.shape[0]).astype(ml_dtypes.bfloat16)
